# revision 1
# baseline (speedup 1.0000x reference)
"""Multi-head self-attention on 8 TRN2 NeuronCores.

Sharding: (batch, head-quad). Core c owns batch c//4 and heads
4*(c%4)..4*(c%4)+3, i.e. a 256-column slice of Wq/Wk/Wv and the matching
256-row slice of Wo. The host sums the 4 partial outputs per batch (the
tensor-parallel all-reduce) and adds the output bias.

Tricks:
- X is augmented with a ones-row (a 1-partition SBUF memset) so the QKV
  biases AND the softmax-normalizer ones-column of V' fall out of the
  projection matmuls: Wv is augmented per-head with a unit column whose
  only nonzero is in the ones-row, so V' = [V_h | 1] per head.
- V' is projected directly into [token, col] layout (lhsT = X chunk), so
  attnV needs no transposes and no tile reassembly.
- Scores for two 512-query chunks land in one 2-bank PSUM tile and are
  exponentiated by a single 1024-wide activation.
- The attention inner loop is one flat (qp, h, kt) pipeline: the next
  score pair is emitted BEFORE the current attnV pair (PE never queues
  behind the exp dependency) and crosses h/qp boundaries; each query
  pair's output projection is drip-fed between the next segment's score
  pairs so the shared PSUM pool rotation alternates sc/po.
- The K projection runs d-outer across 8 PSUM banks so it pipelines with
  the X DMA; accumulating matmuls also warm the PE p-state.
- Softmax normalizer reciprocal rows are broadcast across partitions via
  a DRAM bounce (partition-stride-0 DMA read).

Precision: f16 activations/weights everywhere on the PE; PSUM f32;
partial outputs f32.
"""

import numpy as np

B, S, D, H, DK = 2, 2048, 1024, 16, 64
NCORES = 8
GPB = 4                    # head-groups per batch
HPC = H // GPB             # heads per core = 4
COLS = HPC * DK            # feature columns per core = 256
VCOLS = HPC * (DK + 1)     # V' columns incl per-head ones = 260
ND = D // 128              # 8 contraction chunks
NT = S // 512              # 4 token chunks (free dim 512)
NKT = S // 128             # 16 key tiles
NQP = S // 1024            # 2 query pairs
NE = D // 128              # 8 output-column chunks

_CACHE = {}


def _build_program():
    from contextlib import ExitStack

    import concourse.bacc as bacc
    import concourse.bass as bass
    import concourse.mybir as mybir
    import concourse.tile as tile

    f32 = mybir.dt.float32
    f32r = mybir.dt.float32r
    f16 = mybir.dt.float16
    Exp = mybir.ActivationFunctionType.Exp
    Mult = mybir.AluOpType.mult

    nc = bacc.Bacc("TRN2", target_bir_lowering=False, debug=False,
                   num_devices=NCORES)

    XT = nc.dram_tensor("xt", [D, S], f16, kind="ExternalInput").ap()
    WQ = nc.dram_tensor("wq", [D + 1, COLS], f16, kind="ExternalInput").ap()
    WK = nc.dram_tensor("wk", [D + 1, COLS], f16, kind="ExternalInput").ap()
    WV = nc.dram_tensor("wv", [D + 1, VCOLS], f16, kind="ExternalInput").ap()
    WO = nc.dram_tensor("wo", [COLS, D], f16, kind="ExternalInput").ap()
    OT = nc.dram_tensor("ot", [D, S], f16, kind="ExternalOutput").ap()

    with tile.TileContext(nc) as tc, ExitStack() as ctx:
        consts = ctx.enter_context(tc.tile_pool(name="consts", bufs=1))
        drp = ctx.enter_context(tc.tile_pool(name="drp", bufs=2, space="DRAM"))

        # ---- weights + X chunks ----
        xc = []
        for d in range(ND):
            t = consts.tile([128, S], f16, name=f"xc_{d}")
            nc.gpsimd.dma_start(t, XT[d * 128:(d + 1) * 128, :])
            xc.append(t)

        wq_c, wk_c, wv_c = [], [], []
        for d in range(ND):
            for nm, src, lst, w in (("wq", WQ, wq_c, COLS),
                                    ("wk", WK, wk_c, COLS),
                                    ("wv", WV, wv_c, VCOLS)):
                t = consts.tile([128, w], f16, name=f"{nm}_{d}")
                nc.sync.dma_start(t, src[d * 128:(d + 1) * 128, :])
                lst.append(t)
        wq_b = consts.tile([1, COLS], f16, name="wq_b")
        nc.sync.dma_start(wq_b, WQ[D:D + 1, :])
        wk_b = consts.tile([1, COLS], f16, name="wk_b")
        nc.sync.dma_start(wk_b, WK[D:D + 1, :])
        wv_b = consts.tile([1, VCOLS], f16, name="wv_b")
        nc.sync.dma_start(wv_b, WV[D:D + 1, :])
        wo_sb = []
        for cc in range(2):
            t = consts.tile([128, D], f16, name=f"wo_{cc}")
            nc.sync.dma_start(t, WO[cc * 128:(cc + 1) * 128, :])
            wo_sb.append(t)

        ones_row = consts.tile([1, S], f16, name="ones_row")
        nc.vector.memset(ones_row, 1.0)
        onecol_16 = consts.tile([1, DK], f16, name="onecol")
        nc.vector.memset(onecol_16, 1.0)

        KT = [consts.tile([128, S], f16, name=f"KT_{ct}") for ct in range(2)]
        QT = [consts.tile([128, S], f16, name=f"QT_{ct}") for ct in range(2)]
        VP = [consts.tile([128, VCOLS], f16, name=f"VP_{kt}")
              for kt in range(NKT)]

        # ---- phase 1: projections ----
        with tc.tile_pool(name="pk", bufs=8, space="PSUM") as pk:
            # K proj, d-outer so matmuls consume X chunks as they arrive
            kps = [pk.tile([128, 512], f32, tag="k", name=f"kps_{i}")
                   for i in range(8)]
            for d in range(ND + 1):
                for ct in range(2):
                    cs = slice(ct * 128, (ct + 1) * 128)
                    for tch in range(NT):
                        ts = slice(tch * 512, (tch + 1) * 512)
                        if d < ND:
                            lhsT, rhs = wk_c[d][:, cs], xc[d][:, ts]
                        else:
                            lhsT, rhs = wk_b[:, cs], ones_row[:, ts]
                        nc.tensor.matmul(kps[ct * NT + tch], lhsT=lhsT,
                                         rhs=rhs, start=(d == 0),
                                         stop=(d == ND))
            for ct in range(2):
                for tch in range(NT):
                    ts = slice(tch * 512, (tch + 1) * 512)
                    if (ct * NT + tch) % 2:
                        nc.scalar.copy(KT[ct][:, ts], kps[ct * NT + tch])
                    else:
                        nc.vector.tensor_copy(KT[ct][:, ts],
                                              kps[ct * NT + tch])

        # ---- phases 2+3: flat attention + outproj pipeline ----
        with tc.tile_pool(name="pat", bufs=2, space="PSUM") as pat, \
                tc.tile_pool(name="psc", bufs=2, space="PSUM") as psc, \
                tc.tile_pool(name="expp", bufs=3) as expp, \
                tc.tile_pool(name="attnp", bufs=4) as attnp, \
                tc.tile_pool(name="zp", bufs=4) as zp, \
                tc.tile_pool(name="zbp", bufs=2) as zbp, \
                tc.tile_pool(name="obp", bufs=4) as obp:

            segs = [(qp, h) for qp in range(NQP) for h in range(HPC)]
            attn_tiles = {}
            for qp in range(NQP):
                attn_tiles[qp] = [
                    attnp.tile([128, 1024], f16, tag="attn",
                               name=f"attn_{qp}_{ct}") for ct in range(2)]

            def sc_pair(qp, h, kt):
                ct, hs = h // 2, slice((h % 2) * 64, (h % 2) * 64 + 64)
                qs0 = slice(qp * 1024, qp * 1024 + 512)
                qs1 = slice(qp * 1024 + 512, (qp + 1) * 1024)
                ks = slice(kt * 128, (kt + 1) * 128)
                sc = psc.tile([128, 1024], f32, tag="s",
                              name=f"sc_{qp}_{h}_{kt}")
                nc.tensor.matmul(sc[:, 0:512], lhsT=KT[ct][hs, ks],
                                 rhs=QT[ct][hs, qs0], start=True, stop=True)
                nc.tensor.matmul(sc[:, 512:1024], lhsT=KT[ct][hs, ks],
                                 rhs=QT[ct][hs, qs1], start=True, stop=True)
                return sc

            def q_chain(ct, tch, cast_eng):
                """Project one Q quadrant into a psc tile and cast to QT."""
                cs = slice(ct * 128, (ct + 1) * 128)
                ts = slice(tch * 512, (tch + 1) * 512)
                p = psc.tile([128, 1024], f32, tag="s",
                             name=f"qch_{ct}_{tch}")
                for d in range(ND + 1):
                    if d < ND:
                        lhsT, rhs = wq_c[d][:, cs], xc[d][:, ts]
                    else:
                        lhsT, rhs = wq_b[:, cs], ones_row[:, ts]
                    nc.tensor.matmul(p[:, 0:512], lhsT=lhsT, rhs=rhs,
                                     start=(d == 0), stop=(d == ND))
                if cast_eng == "scalar":
                    nc.scalar.copy(QT[ct][:, ts], p[:, 0:512])
                else:
                    nc.vector.tensor_copy(QT[ct][:, ts], p[:, 0:512])

            def vp_chain(kt):
                """Project one V' token-tile into a psc tile; cast on ACT."""
                ks = slice(kt * 128, (kt + 1) * 128)
                p = psc.tile([128, 1024], f32, tag="s", name=f"vch_{kt}")
                for d in range(ND + 1):
                    if d < ND:
                        lhsT, rhs = xc[d][:, ks], wv_c[d]
                    else:
                        lhsT, rhs = ones_row[:, ks], wv_b
                    nc.tensor.matmul(p[:, 0:VCOLS], lhsT=lhsT, rhs=rhs,
                                     start=(d == 0), stop=(d == ND))
                nc.scalar.copy(VP[kt], p[:, 0:VCOLS])

            def outproj_item(qp, e):
                es = slice(e * 128, (e + 1) * 128)
                po = psc.tile([128, 1024], f32, tag="s", name=f"po_{qp}_{e}")
                for qsl in (slice(0, 512), slice(512, 1024)):
                    for cc in range(2):
                        nc.tensor.matmul(po[:, qsl], lhsT=wo_sb[cc][:, es],
                                         rhs=attn_tiles[qp][cc][:, qsl],
                                         start=(cc == 0), stop=(cc == 1))
                ob = obp.tile([128, 1024], f16, tag="ob", name=f"ob_{qp}_{e}")
                if qp == NQP - 1:
                    # tail: ACT is idle — drain halves on both engines so
                    # the PSUM slot frees at half-drain latency
                    nc.vector.tensor_copy(ob[:, 0:512], po[:, 0:512])
                    nc.scalar.copy(ob[:, 512:1024], po[:, 512:1024])
                else:
                    nc.vector.tensor_copy(ob, po)
                qbase = qp * 1024
                nc.sync.dma_start(OT[es, qbase:qbase + 512], ob[:, 0:512])
                nc.gpsimd.dma_start(OT[es, qbase + 512:qbase + 1024],
                                    ob[:, 512:1024])

            # deferred paired work: (si, kt) -> list of thunks
            drip = {}
            drip[(1, 4)] = [lambda: q_chain(1, 0, "vector"),
                            lambda: q_chain(1, 1, "vector")]
            drip[(2, 4)] = [lambda: q_chain(0, 2, "vector"),
                            lambda: q_chain(0, 3, "vector")]
            drip[(3, 4)] = [lambda: q_chain(1, 2, "vector"),
                            lambda: q_chain(1, 3, "vector")]
            drip[(4, 8)] = [lambda: outproj_item(0, 0),
                            lambda: outproj_item(0, 1)]
            drip[(4, 12)] = [lambda: outproj_item(0, 2),
                             lambda: outproj_item(0, 3)]
            drip[(5, 4)] = [lambda: outproj_item(0, 4),
                            lambda: outproj_item(0, 5)]
            drip[(5, 8)] = [lambda: outproj_item(0, 6),
                            lambda: outproj_item(0, 7)]

            # pre-loop: the two Q quadrants the first segment needs, the
            # first V' tile, then the first score pair
            q_chain(0, 0, "scalar")
            q_chain(0, 1, "scalar")
            vp_chain(0)
            sc = sc_pair(0, 0, 0)
            for si, (qp, h) in enumerate(segs):
                ct, hs = h // 2, slice((h % 2) * 64, (h % 2) * 64 + 64)
                vs = slice(h * (DK + 1), (h + 1) * (DK + 1))
                patt = pat.tile([DK + 1, 1024], f32, tag="p",
                                name=f"patt_{qp}_{h}")
                for kt in range(NKT):
                    ex = expp.tile([128, 1024], f16, tag="e",
                                   name=f"ex_{qp}_{h}_{kt}")
                    nc.scalar.activation(ex, sc, Exp, scale=0.125)
                    # JIT V' for the first segment
                    if si == 0 and kt + 1 < NKT:
                        vp_chain(kt + 1)
                    # prefetch the next score pair (crossing h/qp bounds)
                    if kt + 1 < NKT:
                        sc = sc_pair(qp, h, kt + 1)
                    elif si + 1 < len(segs):
                        sc = sc_pair(segs[si + 1][0], segs[si + 1][1], 0)
                    for job in drip.pop((si, kt), ()):
                        job()
                    nc.tensor.matmul(patt[:, 0:512], lhsT=VP[kt][:, vs],
                                     rhs=ex[:, 0:512], start=(kt == 0),
                                     stop=(kt == NKT - 1))
                    nc.tensor.matmul(patt[:, 512:1024],
                                     lhsT=VP[kt][:, vs],
                                     rhs=ex[:, 512:1024], start=(kt == 0),
                                     stop=(kt == NKT - 1))
                # normalize head h: reciprocal of the ones-column row,
                # broadcast across 64 partitions, multiply
                zs = zp.tile([1, 1024], f32, tag="zs", name=f"zs_{qp}_{h}")
                nc.vector.tensor_copy(zs, patt[DK:DK + 1, :])
                zr = zp.tile([1, 1024], f32, tag="z", name=f"zr_{qp}_{h}")
                nc.vector.reciprocal_approx_fast(zr, zs)
                if si == len(segs) - 1:
                    # tail: no DRAM-bounce latency -- drain attn rows, then
                    # broadcast 1/z via a PE outer product into free PSUM
                    araw = zbp.tile([DK, 1024], f32, tag="zb",
                                    name=f"araw_{qp}_{h}")
                    nc.scalar.copy(araw, patt[0:DK, :])
                    zr16 = zp.tile([1, 1024], f16, tag="z16",
                                   name=f"zr16_{qp}_{h}")
                    nc.vector.tensor_copy(zr16, zr)
                    zbp_ps = psc.tile([128, 1024], f32, tag="s",
                                      name=f"zbps_{qp}_{h}")
                    nc.tensor.matmul(zbp_ps[0:DK, 0:512],
                                     lhsT=onecol_16[:, 0:DK],
                                     rhs=zr16[:, 0:512], start=True,
                                     stop=True)
                    nc.tensor.matmul(zbp_ps[0:DK, 512:1024],
                                     lhsT=onecol_16[:, 0:DK],
                                     rhs=zr16[:, 512:1024],
                                     start=True, stop=True)
                    nc.vector.tensor_tensor(attn_tiles[qp][ct][hs, :],
                                            araw, zbp_ps[0:DK, :], Mult)
                else:
                    scratch = drp.tile([1, 1024], f32, tag="scr",
                                       name=f"scr_{qp}_{h}")
                    nc.gpsimd.dma_start(scratch, zr)
                    zb = zbp.tile([DK, 1024], f32, tag="zb",
                                  name=f"zb_{qp}_{h}")
                    row = scratch[0:1, :]
                    nc.gpsimd.dma_start(
                        zb,
                        bass.AP(tensor=row.tensor, offset=row.offset,
                                ap=[[0, DK]] + row.ap[1:]))
                    nc.vector.tensor_tensor(attn_tiles[qp][ct][hs, :],
                                            patt[0:DK, :], zb, Mult)
            # flush the last query pair's outproj
            for e in range(NE):
                outproj_item(NQP - 1, e)

    nc.compile()
    return nc


def _get_program():
    if "nc" not in _CACHE:
        _CACHE["nc"] = _build_program()
    return _CACHE["nc"]


def _install_ntff_hook():
    """Provide the antenv.axon_hooks shim this container's antenv lacks so
    run_bass_kernel_spmd(trace=True) can capture NTFF profiles."""
    import sys
    import types

    try:
        import antenv

        if hasattr(antenv, "axon_hooks"):
            return
        mod = types.ModuleType("antenv.axon_hooks")
        mod._hook = None
        mod.set_axon_ntff_profile_hook = lambda h: setattr(mod, "_hook", h)
        mod.get_axon_ntff_profile_hook = lambda: mod._hook
        sys.modules["antenv.axon_hooks"] = mod
        antenv.axon_hooks = mod
        from trn_agent_boot.trn_boot import _ntff_profile_via_ctypes

        mod.set_axon_ntff_profile_hook(
            _ntff_profile_via_ctypes("/opt/axon/libaxon_pjrt.so"))
    except Exception:
        pass


def kernel(X, Wq, bq, Wk, bk, Wv, bv, Wo, bo, _profile=False, _trace_cores=None):
    from concourse.bass_utils import run_bass_kernel_spmd

    if _profile:
        _install_ntff_hook()

    nc = _get_program()

    X = np.asarray(X, np.float32)
    Wq, Wk, Wv, Wo = (np.asarray(w, np.float32) for w in (Wq, Wk, Wv, Wo))
    bq, bk, bv, bo = (np.asarray(v, np.float32) for v in (bq, bk, bv, bo))

    in_maps = []
    for c in range(NCORES):
        b, g = c // GPB, c % GPB
        cs = slice(g * COLS, (g + 1) * COLS)
        wq_aug = np.vstack([Wq[:, cs], bq[cs][None, :]])
        wk_aug = np.vstack([Wk[:, cs], bk[cs][None, :]])
        wv_aug = np.zeros((D + 1, VCOLS), np.float32)
        for h in range(HPC):
            hc = slice(g * COLS + h * DK, g * COLS + (h + 1) * DK)
            wv_aug[:D, h * (DK + 1):h * (DK + 1) + DK] = Wv[:, hc]
            wv_aug[D, h * (DK + 1):h * (DK + 1) + DK] = bv[hc]
            wv_aug[D, h * (DK + 1) + DK] = 1.0
        in_maps.append({
            "xt": np.ascontiguousarray(X[b].T).astype(np.float16),
            "wq": wq_aug.astype(np.float16),
            "wk": wk_aug.astype(np.float16),
            "wv": wv_aug.astype(np.float16),
            "wo": np.ascontiguousarray(Wo[cs, :]).astype(np.float16),
        })

    res = run_bass_kernel_spmd(
        nc, in_maps, core_ids=list(range(NCORES)),
        trace=_profile,
        trace_cores=(_trace_cores if _trace_cores is not None
                     else ([0] if _profile else None)),
    )

    out = np.empty((B, S, D), np.float32)
    for b in range(B):
        ot = res.results[b * GPB]["ot"].astype(np.float64)
        for g in range(1, GPB):
            ot += res.results[b * GPB + g]["ot"]
        out[b] = (ot.T + bo).astype(np.float32)
    if _profile:
        kernel.last_exec_time_ns = res.exec_time_ns
        kernel.last_results = res
    return out

